# revision 18
# baseline (speedup 1.0000x reference)
"""v5: (16,32)-factored histogram, per-row tensor_scalar one-hots, tri-engine
split, flipped matmul.

Binning: Act engine emits int16 planes t_c = rne(2*x_c + 4.5) + 1024 (fp32->
int16 output conversion is round-nearest-even). Valid digits q' = t-1024 in
[1..8]; tails never match any target (max |x| = 5.42 < 6.25 where the first
collision window opens).

Factors (per point): hi = 2*(t0-1024) + b1 with b1 = [t1 >= 1029], 16 targets;
lo = 16*(t1-1024) + t2 - 64*b1, 32 targets (values 1024 + 16*q1l + q2'),
all exact in fp16. One-hot rows are built per target with tensor_scalar
is_equal (DVE runs it in 4x mode, 0.26 ns/elem vs tensor_tensor's 2x):
- DVE: all 16 hi rows + lo rows [0, LO_POOL)
- Pool: lo rows [LO_POOL, LO_ACT) via ts is_equal (strict)
- Act:  lo rows [LO_ACT, 32) via Square+Sign, giving +-1 rows (hit=+1,
  miss/-invalid=-1). A constant ones-row (33rd stationary column) lets the
  host recover counts exactly: C = (M_row + M_ones)/2, folded into the
  stage-2 weights. Invalid points cancel exactly in this algebra.

Stage-1 matmul: 33-wide lo factor as stationary LDWEIGHTS, 16-wide hi moving
-> out free 16. PSUM [33, 32, 16] per bank; one fp16 copy per 32 batches.
Stage-2: 16 W-stationary matmuls contracting the 33 lo rows.
"""

import numpy as np

B, N, VR, CLS = 1024, 8192, 8, 40
NCORES = 8
BPC = B // NCORES
PJ = N // 128
G = 8            # batches per work group
HI_W, LO_W = 16, 32
LO_POOL = 17     # lo rows [LO_POOL, LO_ACT) on gpsimd
LO_ACT = 27      # lo rows [LO_ACT, LO_W) on activation engine (+- form)

HI_VALS = [2 * (ih // 2 + 1) + (ih % 2) for ih in range(HI_W)]
LO_VALS = [1024 + 16 * (il // 8 + 1) + (il % 8 + 1) for il in range(LO_W)]

_CACHE = {}


def _build(n_batches):
    import concourse.bacc as bacc
    import concourse.mybir as mybir
    import concourse.tile as tile

    dt = mybir.dt
    op = mybir.AluOpType
    AF = mybir.ActivationFunctionType
    nc = bacc.Bacc("TRN2", target_bir_lowering=False, debug=False,
                   num_devices=NCORES)

    ngrp = n_batches // G
    x_d = nc.dram_tensor("x", (128, n_batches, PJ, 3), dt.float32,
                         kind="ExternalInput")
    w2_d = nc.dram_tensor("w2", (LO_W + 1, 2 * HI_W * CLS), dt.float16,
                          kind="ExternalInput")
    bias_d = nc.dram_tensor("bias", (CLS, 1), dt.float32,
                            kind="ExternalInput")
    actb_d = nc.dram_tensor("actb", (128, LO_W - LO_ACT + 1), dt.float32,
                            kind="ExternalInput")
    y_d = nc.dram_tensor("y", (CLS, n_batches), dt.float32,
                         kind="ExternalOutput")

    SPAN = 32
    nspan = n_batches // SPAN

    with tile.TileContext(nc) as tc:
        with (
            tc.tile_pool(name="const", bufs=1) as cpool,
            tc.tile_pool(name="work", bufs=2) as wpool,
            tc.tile_pool(name="oh", bufs=2) as ohpool,
            tc.tile_pool(name="cnt", bufs=1) as cntpool,
            tc.tile_pool(name="ps1", bufs=2, space="PSUM") as ps1pool,
            tc.tile_pool(name="ps2", bufs=1, space="PSUM") as ps2pool,
        ):
            actb = cpool.tile([128, LO_W - LO_ACT + 1], dt.float32)
            nc.sync.dma_start(actb[:], actb_d[:])

            cnt = cntpool.tile([LO_W + 1, n_batches, HI_W], dt.float16)
            # static double-buffered lo one-hot tiles so the constant
            # ones-row (33rd stationary column) is written only once
            lo_oh_bufs = [
                cntpool.tile([128, G, LO_W + 1, PJ], dt.float16,
                             name=f"lo_oh{i}")
                for i in range(2)
            ]
            for t in lo_oh_bufs:
                nc.gpsimd.memset(t[:, :, LO_W, :], 1.0)

            ps1 = [None] * nspan
            pending = []

            def _flush(item):
                gg, lo_g, lo_oh_g, hi_oh_g, use_pm = item
                for v in range(LO_ACT, LO_W) if use_pm else ():
                    sq = wpool.tile([128, G, PJ], dt.float16, tag="sq",
                                    name=f"sq_{gg}_{v}")
                    nc.scalar.activation(sq[:], lo_g[:], AF.Square,
                                         bias=actb[:, v - LO_ACT:v - LO_ACT + 1],
                                         scale=1.0)
                    nc.scalar.activation(lo_oh_g[:, :, v, :], sq[:], AF.Sign,
                                         bias=actb[:, LO_W - LO_ACT:LO_W - LO_ACT + 1],
                                         scale=-1.0)
                for bb in range(G):
                    b = gg * G + bb
                    sp = b // SPAN
                    if ps1[sp] is None:
                        ps1[sp] = ps1pool.tile([LO_W + 1, SPAN, HI_W],
                                               dt.float32,
                                               tag=f"ps1_{sp % 2}",
                                               name=f"ps1_{sp}")
                    for j in range(PJ):
                        nc.tensor.matmul(ps1[sp][:, b % SPAN, :],
                                         lo_oh_g[:, bb, :, j],
                                         hi_oh_g[:, bb, :, j],
                                         start=(j == 0), stop=(j == PJ - 1))
                    if b % SPAN == SPAN - 1:
                        nc.scalar.copy(cnt[:, sp * SPAN:(sp + 1) * SPAN, :],
                                       ps1[sp][:])
                        ps1[sp] = None

            for g in range(ngrp):
                xg = wpool.tile([128, G, PJ, 3], dt.float32, tag="xg")
                nc.sync.dma_start(xg[:], x_d[:, g * G:(g + 1) * G])

                t0 = wpool.tile([128, G, PJ], dt.int16, tag="t0")
                nc.scalar.activation(t0[:], xg[:, :, :, 0], AF.Copy,
                                     bias=1028.5, scale=2.0)
                t1 = wpool.tile([128, G, PJ], dt.int16, tag="t1")
                nc.scalar.activation(t1[:], xg[:, :, :, 1], AF.Copy,
                                     bias=1028.5, scale=2.0)
                t2 = wpool.tile([128, G, PJ], dt.int16, tag="t2")
                nc.scalar.activation(t2[:], xg[:, :, :, 2], AF.Copy,
                                     bias=1028.5, scale=2.0)

                b1 = wpool.tile([128, G, PJ], dt.float16, tag="b1")
                nc.vector.tensor_scalar(b1[:], t1[:], 1029, None, op.is_ge)
                hi = wpool.tile([128, G, PJ], dt.float16, tag="hi")
                nc.vector.tensor_scalar(hi[:], t0[:], 1024, 2, op.subtract,
                                        op.mult)
                nc.vector.tensor_tensor(hi[:], hi[:], b1[:], op.add)
                u = wpool.tile([128, G, PJ], dt.float16, tag="u")
                nc.vector.tensor_scalar(u[:], t1[:], 1024, 16, op.subtract,
                                        op.mult)
                b64 = wpool.tile([128, G, PJ], dt.float16, tag="b64")
                nc.vector.tensor_scalar(b64[:], b1[:], 64, None, op.mult)
                lo = wpool.tile([128, G, PJ], dt.float16, tag="lo")
                nc.vector.tensor_tensor(lo[:], u[:], t2[:], op.add)
                nc.vector.tensor_tensor(lo[:], lo[:], b64[:], op.subtract)

                hi_oh = ohpool.tile([128, G, HI_W, PJ], dt.float16,
                                    tag="hi_oh")
                for v in range(HI_W):
                    nc.vector.tensor_scalar(hi_oh[:, :, v, :], hi[:],
                                            float(HI_VALS[v]), None,
                                            op.is_equal)
                lo_oh = lo_oh_bufs[g % 2]
                last = g == ngrp - 1
                for v in range(LO_POOL):
                    nc.vector.tensor_scalar(lo_oh[:, :, v, :], lo[:],
                                            float(LO_VALS[v]), None,
                                            op.is_equal)
                for v in range(LO_POOL, LO_ACT):
                    eng = nc.vector if last else nc.gpsimd
                    eng.tensor_scalar(lo_oh[:, :, v, :], lo[:],
                                      float(LO_VALS[v]), None,
                                      op.is_equal)
                if last:
                    for v in range(LO_ACT, LO_W):
                        nc.vector.tensor_scalar(lo_oh[:, :, v, :], lo[:],
                                                float(LO_VALS[v]), None,
                                                op.is_equal)
                pending.append((g, lo, lo_oh, hi_oh, not last))
                if len(pending) > 1:
                    _flush(pending.pop(0))

            while pending:
                _flush(pending.pop(0))

            w2 = cpool.tile([LO_W + 1, 2, HI_W, CLS], dt.float16)
            nc.sync.dma_start(
                w2[:], w2_d.ap().rearrange("p (v m c) -> p v m c", v=2,
                                           m=HI_W))
            bias = cpool.tile([CLS, 1], dt.float32)
            nc.sync.dma_start(bias[:], bias_d[:])
            ps2 = ps2pool.tile([CLS, n_batches], dt.float32)
            nb0 = n_batches - G
            for h in range(HI_W):
                nc.tensor.matmul(ps2[:, :nb0], w2[:, 0, h, :],
                                 cnt[:, :nb0, h],
                                 start=(h == 0), stop=(h == HI_W - 1))
            for h in range(HI_W):
                nc.tensor.matmul(ps2[:, nb0:], w2[:, 1, h, :],
                                 cnt[:, nb0:, h],
                                 start=(h == 0), stop=(h == HI_W - 1))
            out = cpool.tile([CLS, n_batches], dt.float32)
            nc.vector.tensor_scalar(out[:], ps2[:], 1.0 / N, bias[:],
                                    op.mult, op.add)
            nc.sync.dma_start(y_d[:], out[:])

    nc.compile()
    return nc


def _aux_inputs(W, b):
    # bin(l, h) = 64*q0 + 8*q1 + q2 with q0 = h>>1, q1 = 4*(h&1) + l//8,
    # q2 = l % 8
    ih = np.arange(HI_W)
    il = np.arange(LO_W)
    q0 = ih[:, None] // 2
    b1 = ih[:, None] % 2
    q1 = 4 * b1 + il[None, :] // 8
    q2 = il[None, :] % 8
    binidx = 64 * q0 + 8 * q1 + q2            # [HI_W, LO_W]
    wmap = W[:, binidx]                       # [CLS, HI_W, LO_W]
    w2 = np.zeros((LO_W + 1, 2, HI_W, CLS), np.float32)
    for l in range(LO_W):
        scale = 0.5 if l >= LO_ACT else 1.0
        w2[l, 0] = scale * wmap[:, :, l].T
        w2[l, 1] = wmap[:, :, l].T            # strict last group
    # ones-row: sum over +- rows of W/2 (variant 0 only)
    w2[LO_W, 0] = 0.5 * wmap[:, :, LO_ACT:].sum(-1).T
    w2 = np.ascontiguousarray(w2).astype(np.float16).reshape(
        LO_W + 1, 2 * HI_W * CLS)
    bias = np.asarray(b, dtype=np.float32).reshape(CLS, 1)
    # act biases: -lo_val for the +- rows, then +0.5 for the Sign pass
    actb = np.zeros((128, LO_W - LO_ACT + 1), np.float32)
    for v in range(LO_ACT, LO_W):
        actb[:, v - LO_ACT] = -float(LO_VALS[v])
    actb[:, LO_W - LO_ACT] = 0.5
    return w2, bias, actb


def kernel(x, W, b):
    from concourse.bass_utils import run_bass_kernel_spmd

    x = np.asarray(x, dtype=np.float32)
    W = np.asarray(W, dtype=np.float32)
    b = np.asarray(b, dtype=np.float32)

    if BPC not in _CACHE:
        _CACHE[BPC] = _build(BPC)
    nc = _CACHE[BPC]

    w2, bias, actb = _aux_inputs(W, b)
    shards = x.reshape(NCORES, BPC, 128, PJ, 3).transpose(0, 2, 1, 3, 4)
    in_maps = [
        {"x": np.ascontiguousarray(shards[i]), "w2": w2, "bias": bias,
         "actb": actb}
        for i in range(NCORES)
    ]
    res = run_bass_kernel_spmd(nc, in_maps, list(range(NCORES)))
    return np.concatenate(
        [np.asarray(res.results[i]["y"]).T for i in range(NCORES)],
        axis=0).astype(np.float32)


# revision 20
# speedup vs baseline: 1.0142x; 1.0142x over previous
"""v5: (16,32)-factored histogram, per-row tensor_scalar one-hots, tri-engine
split, flipped matmul.

Binning: Act engine emits int16 planes t_c = rne(2*x_c + 4.5) + 1024 (fp32->
int16 output conversion is round-nearest-even). Valid digits q' = t-1024 in
[1..8]; tails never match any target (max |x| = 5.42 < 6.25 where the first
collision window opens).

Factors (per point): hi = 2*(t0-1024) + b1 with b1 = [t1 >= 1029], 16 targets;
lo = 16*(t1-1024) + t2 - 64*b1, 32 targets (values 1024 + 16*q1l + q2'),
all exact in fp16. One-hot rows are built per target with tensor_scalar
is_equal (DVE runs it in 4x mode, 0.26 ns/elem vs tensor_tensor's 2x):
- DVE: all 16 hi rows + lo rows [0, LO_POOL)
- Pool: lo rows [LO_POOL, LO_ACT) via ts is_equal (strict)
- Act:  lo rows [LO_ACT, 32) via Square+Sign, giving +-1 rows (hit=+1,
  miss/-invalid=-1). A constant ones-row (33rd stationary column) lets the
  host recover counts exactly: C = (M_row + M_ones)/2, folded into the
  stage-2 weights. Invalid points cancel exactly in this algebra.

Stage-1 matmul: 33-wide lo factor as stationary LDWEIGHTS, 16-wide hi moving
-> out free 16. PSUM [33, 32, 16] per bank; one fp16 copy per 32 batches.
Stage-2: 16 W-stationary matmuls contracting the 33 lo rows.
"""

import numpy as np

B, N, VR, CLS = 1024, 8192, 8, 40
NCORES = 8
BPC = B // NCORES
PJ = N // 128
G = 8            # batches per work group
HI_W, LO_W = 16, 32
LO_POOL = 17     # lo rows [LO_POOL, LO_ACT) on gpsimd
LO_ACT = 27      # lo rows [LO_ACT, LO_W) on activation engine (+- form)

HI_VALS = [2 * (ih // 2 + 1) + (ih % 2) for ih in range(HI_W)]
LO_VALS = [1024 + 16 * (il // 8 + 1) + (il % 8 + 1) for il in range(LO_W)]

_CACHE = {}


def _build(n_batches):
    import concourse.bacc as bacc
    import concourse.mybir as mybir
    import concourse.tile as tile

    dt = mybir.dt
    op = mybir.AluOpType
    AF = mybir.ActivationFunctionType
    nc = bacc.Bacc("TRN2", target_bir_lowering=False, debug=False,
                   num_devices=NCORES)

    ngrp = n_batches // G
    x_d = nc.dram_tensor("x", (128, n_batches, PJ, 3), dt.float32,
                         kind="ExternalInput")
    w2_d = nc.dram_tensor("w2", (LO_W + 1, HI_W * CLS), dt.float16,
                          kind="ExternalInput")
    bias_d = nc.dram_tensor("bias", (CLS, 1), dt.float32,
                            kind="ExternalInput")
    actb_d = nc.dram_tensor("actb", (128, LO_W - LO_ACT + 1), dt.float32,
                            kind="ExternalInput")
    y_d = nc.dram_tensor("y", (CLS, n_batches), dt.float32,
                         kind="ExternalOutput")

    SPAN = 32
    nspan = n_batches // SPAN

    with tile.TileContext(nc) as tc:
        with (
            tc.tile_pool(name="const", bufs=1) as cpool,
            tc.tile_pool(name="work", bufs=2) as wpool,
            tc.tile_pool(name="oh", bufs=2) as ohpool,
            tc.tile_pool(name="cnt", bufs=1) as cntpool,
            tc.tile_pool(name="ps1", bufs=2, space="PSUM") as ps1pool,
            tc.tile_pool(name="ps2", bufs=1, space="PSUM") as ps2pool,
        ):
            actb = cpool.tile([128, LO_W - LO_ACT + 1], dt.float32)

            cnt = cntpool.tile([LO_W + 1, n_batches, HI_W], dt.float16)
            # static double-buffered lo one-hot tiles so the constant
            # ones-row (33rd stationary column) is written only once
            lo_oh_bufs = [
                cntpool.tile([128, G, LO_W + 1, PJ], dt.float16,
                             name=f"lo_oh{i}")
                for i in range(2)
            ]
            for t in lo_oh_bufs:
                nc.gpsimd.memset(t[:, :, LO_W, :], 1.0)

            ps1 = [None] * nspan
            pending = []

            def _flush(item):
                gg, lo_g, lo_oh_g, hi_oh_g = item
                for v in range(LO_ACT, LO_W):
                    sq = wpool.tile([128, G, PJ], dt.float16, tag="sq",
                                    name=f"sq_{gg}_{v}")
                    nc.scalar.activation(sq[:], lo_g[:], AF.Square,
                                         bias=actb[:, v - LO_ACT:v - LO_ACT + 1],
                                         scale=1.0)
                    nc.scalar.activation(lo_oh_g[:, :, v, :], sq[:], AF.Sign,
                                         bias=actb[:, LO_W - LO_ACT:LO_W - LO_ACT + 1],
                                         scale=-1.0)
                for bb in range(G):
                    b = gg * G + bb
                    sp = b // SPAN
                    if ps1[sp] is None:
                        ps1[sp] = ps1pool.tile([LO_W + 1, SPAN, HI_W],
                                               dt.float32,
                                               tag=f"ps1_{sp % 2}",
                                               name=f"ps1_{sp}")
                    for j in range(PJ):
                        nc.tensor.matmul(ps1[sp][:, b % SPAN, :],
                                         lo_oh_g[:, bb, :, j],
                                         hi_oh_g[:, bb, :, j],
                                         start=(j == 0), stop=(j == PJ - 1))
                    if b % SPAN == SPAN - 1:
                        nc.scalar.copy(cnt[:, sp * SPAN:(sp + 1) * SPAN, :],
                                       ps1[sp][:])
                        ps1[sp] = None

            for g in range(ngrp):
                xg = wpool.tile([128, G, PJ, 3], dt.float32, tag="xg")
                nc.sync.dma_start(xg[:], x_d[:, g * G:(g + 1) * G])
                if g == 0:
                    nc.sync.dma_start(actb[:], actb_d[:])

                t1 = wpool.tile([128, G, PJ], dt.int16, tag="t1")
                nc.scalar.activation(t1[:], xg[:, :, :, 1], AF.Copy,
                                     bias=1028.5, scale=2.0)
                t0 = wpool.tile([128, G, PJ], dt.int16, tag="t0")
                nc.scalar.activation(t0[:], xg[:, :, :, 0], AF.Copy,
                                     bias=1028.5, scale=2.0)
                t2 = wpool.tile([128, G, PJ], dt.int16, tag="t2")
                nc.scalar.activation(t2[:], xg[:, :, :, 2], AF.Copy,
                                     bias=1028.5, scale=2.0)

                b1 = wpool.tile([128, G, PJ], dt.float16, tag="b1")
                nc.vector.tensor_scalar(b1[:], t1[:], 1029, None, op.is_ge)
                u = wpool.tile([128, G, PJ], dt.float16, tag="u")
                nc.vector.tensor_scalar(u[:], t1[:], 1024, 16, op.subtract,
                                        op.mult)
                b64 = wpool.tile([128, G, PJ], dt.float16, tag="b64")
                nc.vector.tensor_scalar(b64[:], b1[:], 64, None, op.mult)
                hi = wpool.tile([128, G, PJ], dt.float16, tag="hi")
                nc.vector.tensor_scalar(hi[:], t0[:], 1024, 2, op.subtract,
                                        op.mult)
                nc.vector.tensor_tensor(hi[:], hi[:], b1[:], op.add)
                lo = wpool.tile([128, G, PJ], dt.float16, tag="lo")
                nc.vector.tensor_tensor(lo[:], u[:], t2[:], op.add)
                nc.vector.tensor_tensor(lo[:], lo[:], b64[:], op.subtract)

                hi_oh = ohpool.tile([128, G, HI_W, PJ], dt.float16,
                                    tag="hi_oh")
                for v in range(HI_W):
                    nc.vector.tensor_scalar(hi_oh[:, :, v, :], hi[:],
                                            float(HI_VALS[v]), None,
                                            op.is_equal)
                lo_oh = lo_oh_bufs[g % 2]
                for v in range(LO_POOL):
                    nc.vector.tensor_scalar(lo_oh[:, :, v, :], lo[:],
                                            float(LO_VALS[v]), None,
                                            op.is_equal)
                for v in range(LO_POOL, LO_ACT):
                    nc.gpsimd.tensor_scalar(lo_oh[:, :, v, :], lo[:],
                                            float(LO_VALS[v]), None,
                                            op.is_equal)
                pending.append((g, lo, lo_oh, hi_oh))
                if len(pending) > 1:
                    _flush(pending.pop(0))

            while pending:
                _flush(pending.pop(0))

            w2 = cpool.tile([LO_W + 1, HI_W, CLS], dt.float16)
            nc.sync.dma_start(
                w2[:], w2_d.ap().rearrange("p (m c) -> p m c", m=HI_W))
            bias = cpool.tile([CLS, 1], dt.float32)
            nc.sync.dma_start(bias[:], bias_d[:])
            ps2 = ps2pool.tile([CLS, n_batches], dt.float32)
            for h in range(HI_W):
                nc.tensor.matmul(ps2[:], w2[:, h, :], cnt[:, :, h],
                                 start=(h == 0), stop=(h == HI_W - 1))
            out = cpool.tile([CLS, n_batches], dt.float32)
            nc.vector.tensor_scalar(out[:], ps2[:], 1.0 / N, bias[:],
                                    op.mult, op.add)
            nc.sync.dma_start(y_d[:], out[:])

    nc.compile()
    return nc


def _aux_inputs(W, b):
    # bin(l, h) = 64*q0 + 8*q1 + q2 with q0 = h>>1, q1 = 4*(h&1) + l//8,
    # q2 = l % 8
    ih = np.arange(HI_W)
    il = np.arange(LO_W)
    q0 = ih[:, None] // 2
    b1 = ih[:, None] % 2
    q1 = 4 * b1 + il[None, :] // 8
    q2 = il[None, :] % 8
    binidx = 64 * q0 + 8 * q1 + q2            # [HI_W, LO_W]
    wmap = W[:, binidx]                       # [CLS, HI_W, LO_W]
    w2 = np.zeros((LO_W + 1, HI_W, CLS), np.float32)
    for l in range(LO_W):
        scale = 0.5 if l >= LO_ACT else 1.0
        w2[l] = scale * wmap[:, :, l].T
    # ones-row: sum over +- rows of W/2
    w2[LO_W] = 0.5 * wmap[:, :, LO_ACT:].sum(-1).T
    w2 = np.ascontiguousarray(w2).astype(np.float16).reshape(
        LO_W + 1, HI_W * CLS)
    bias = np.asarray(b, dtype=np.float32).reshape(CLS, 1)
    # act biases: -lo_val for the +- rows, then +0.5 for the Sign pass
    actb = np.zeros((128, LO_W - LO_ACT + 1), np.float32)
    for v in range(LO_ACT, LO_W):
        actb[:, v - LO_ACT] = -float(LO_VALS[v])
    actb[:, LO_W - LO_ACT] = 0.5
    return w2, bias, actb


def kernel(x, W, b):
    from concourse.bass_utils import run_bass_kernel_spmd

    x = np.asarray(x, dtype=np.float32)
    W = np.asarray(W, dtype=np.float32)
    b = np.asarray(b, dtype=np.float32)

    if BPC not in _CACHE:
        _CACHE[BPC] = _build(BPC)
    nc = _CACHE[BPC]

    w2, bias, actb = _aux_inputs(W, b)
    shards = x.reshape(NCORES, BPC, 128, PJ, 3).transpose(0, 2, 1, 3, 4)
    in_maps = [
        {"x": np.ascontiguousarray(shards[i]), "w2": w2, "bias": bias,
         "actb": actb}
        for i in range(NCORES)
    ]
    res = run_bass_kernel_spmd(nc, in_maps, list(range(NCORES)))
    return np.concatenate(
        [np.asarray(res.results[i]["y"]).T for i in range(NCORES)],
        axis=0).astype(np.float32)


# revision 21
# speedup vs baseline: 1.0176x; 1.0033x over previous
"""v5: (16,32)-factored histogram, per-row tensor_scalar one-hots, tri-engine
split, flipped matmul.

Binning: Act engine emits int16 planes t_c = rne(2*x_c + 4.5) + 1024 (fp32->
int16 output conversion is round-nearest-even). Valid digits q' = t-1024 in
[1..8]; tails never match any target (max |x| = 5.42 < 6.25 where the first
collision window opens).

Factors (per point): hi = 2*(t0-1024) + b1 with b1 = [t1 >= 1029], 16 targets;
lo = 16*(t1-1024) + t2 - 64*b1, 32 targets (values 1024 + 16*q1l + q2'),
all exact in fp16. One-hot rows are built per target with tensor_scalar
is_equal (DVE runs it in 4x mode, 0.26 ns/elem vs tensor_tensor's 2x):
- DVE: all 16 hi rows + lo rows [0, LO_POOL)
- Pool: lo rows [LO_POOL, LO_ACT) via ts is_equal (strict)
- Act:  lo rows [LO_ACT, 32) via Square+Sign, giving +-1 rows (hit=+1,
  miss/-invalid=-1). A constant ones-row (33rd stationary column) lets the
  host recover counts exactly: C = (M_row + M_ones)/2, folded into the
  stage-2 weights. Invalid points cancel exactly in this algebra.

Stage-1 matmul: 33-wide lo factor as stationary LDWEIGHTS, 16-wide hi moving
-> out free 16. PSUM [33, 32, 16] per bank; one fp16 copy per 32 batches.
Stage-2: 16 W-stationary matmuls contracting the 33 lo rows.
"""

import numpy as np

B, N, VR, CLS = 1024, 8192, 8, 40
NCORES = 8
BPC = B // NCORES
PJ = N // 128
G = 8            # batches per work group
HI_W, LO_W = 16, 32
LO_POOL = 17     # lo rows [LO_POOL, LO_ACT) on gpsimd
LO_ACT = 27      # lo rows [LO_ACT, LO_W) on activation engine (+- form)

HI_VALS = [2 * (ih // 2 + 1) + (ih % 2) for ih in range(HI_W)]
LO_VALS = [1024 + 16 * (il // 8 + 1) + (il % 8 + 1) for il in range(LO_W)]

_CACHE = {}


def _build(n_batches):
    import concourse.bacc as bacc
    import concourse.mybir as mybir
    import concourse.tile as tile

    dt = mybir.dt
    op = mybir.AluOpType
    AF = mybir.ActivationFunctionType
    nc = bacc.Bacc("TRN2", target_bir_lowering=False, debug=False,
                   num_devices=NCORES)

    ngrp = n_batches // G
    x_d = nc.dram_tensor("x", (128, n_batches, PJ, 3), dt.float32,
                         kind="ExternalInput")
    w2_d = nc.dram_tensor("w2", (LO_W + 1, HI_W * CLS), dt.float16,
                          kind="ExternalInput")
    bias_d = nc.dram_tensor("bias", (CLS, 1), dt.float32,
                            kind="ExternalInput")
    actb_d = nc.dram_tensor("actb", (128, LO_W - LO_ACT + 1), dt.float32,
                            kind="ExternalInput")
    y_d = nc.dram_tensor("y", (CLS, n_batches), dt.float32,
                         kind="ExternalOutput")

    SPAN = 32
    nspan = n_batches // SPAN

    with tile.TileContext(nc) as tc:
        with (
            tc.tile_pool(name="const", bufs=1) as cpool,
            tc.tile_pool(name="work", bufs=2) as wpool,
            tc.tile_pool(name="oh", bufs=2) as ohpool,
            tc.tile_pool(name="cnt", bufs=1) as cntpool,
            tc.tile_pool(name="ps1", bufs=2, space="PSUM") as ps1pool,
            tc.tile_pool(name="ps2", bufs=1, space="PSUM") as ps2pool,
        ):
            actb = cpool.tile([128, LO_W - LO_ACT + 1], dt.float32)
            nc.sync.dma_start(actb[:], actb_d[:])

            cnt = cntpool.tile([LO_W + 1, n_batches, HI_W], dt.float16)
            # static double-buffered lo one-hot tiles so the constant
            # ones-row (33rd stationary column) is written only once
            lo_oh_bufs = [
                cntpool.tile([128, G, LO_W + 1, PJ], dt.float16,
                             name=f"lo_oh{i}")
                for i in range(2)
            ]
            for t in lo_oh_bufs:
                nc.gpsimd.memset(t[:, :, LO_W, :], 1.0)

            ps1 = [None] * nspan
            pending = []

            def _flush(item):
                gg, lo_g, lo_oh_g, hi_oh_g = item
                for v in range(LO_ACT, LO_W):
                    sq = wpool.tile([128, G, PJ], dt.float16, tag="sq",
                                    name=f"sq_{gg}_{v}")
                    nc.scalar.activation(sq[:], lo_g[:], AF.Square,
                                         bias=actb[:, v - LO_ACT:v - LO_ACT + 1],
                                         scale=1.0)
                    nc.scalar.activation(lo_oh_g[:, :, v, :], sq[:], AF.Sign,
                                         bias=actb[:, LO_W - LO_ACT:LO_W - LO_ACT + 1],
                                         scale=-1.0)
                for bb in range(G):
                    b = gg * G + bb
                    sp = b // SPAN
                    if ps1[sp] is None:
                        ps1[sp] = ps1pool.tile([LO_W + 1, SPAN, HI_W],
                                               dt.float32,
                                               tag=f"ps1_{sp % 2}",
                                               name=f"ps1_{sp}")
                    for j in range(PJ):
                        nc.tensor.matmul(ps1[sp][:, b % SPAN, :],
                                         lo_oh_g[:, bb, :, j],
                                         hi_oh_g[:, bb, :, j],
                                         start=(j == 0), stop=(j == PJ - 1))
                    if b % SPAN == SPAN - 1:
                        nc.scalar.copy(cnt[:, sp * SPAN:(sp + 1) * SPAN, :],
                                       ps1[sp][:])
                        ps1[sp] = None

            for g in range(ngrp):
                xg = wpool.tile([128, G, PJ, 3], dt.float32, tag="xg")
                nc.sync.dma_start(xg[:], x_d[:, g * G:(g + 1) * G])

                t0 = wpool.tile([128, G, PJ], dt.int16, tag="t0")
                nc.scalar.activation(t0[:], xg[:, :, :, 0], AF.Copy,
                                     bias=1028.5, scale=2.0)
                t1 = wpool.tile([128, G, PJ], dt.int16, tag="t1")
                nc.scalar.activation(t1[:], xg[:, :, :, 1], AF.Copy,
                                     bias=1028.5, scale=2.0)
                t2 = wpool.tile([128, G, PJ], dt.int16, tag="t2")
                nc.scalar.activation(t2[:], xg[:, :, :, 2], AF.Copy,
                                     bias=1028.5, scale=2.0)

                b1 = wpool.tile([128, G, PJ], dt.float16, tag="b1")
                nc.vector.tensor_scalar(b1[:], t1[:], 1029, None, op.is_ge)
                hi = wpool.tile([128, G, PJ], dt.float16, tag="hi")
                nc.vector.tensor_scalar(hi[:], t0[:], 1024, 2, op.subtract,
                                        op.mult)
                nc.vector.tensor_tensor(hi[:], hi[:], b1[:], op.add)
                u = wpool.tile([128, G, PJ], dt.float16, tag="u")
                nc.vector.tensor_scalar(u[:], t1[:], 1024, 16, op.subtract,
                                        op.mult)
                b64 = wpool.tile([128, G, PJ], dt.float16, tag="b64")
                nc.vector.tensor_scalar(b64[:], b1[:], 64, None, op.mult)
                lo = wpool.tile([128, G, PJ], dt.float16, tag="lo")
                nc.vector.tensor_tensor(lo[:], u[:], t2[:], op.add)
                nc.vector.tensor_tensor(lo[:], lo[:], b64[:], op.subtract)

                hi_oh = ohpool.tile([128, G, HI_W, PJ], dt.float16,
                                    tag="hi_oh")
                for v in range(HI_W):
                    nc.vector.tensor_scalar(hi_oh[:, :, v, :], hi[:],
                                            float(HI_VALS[v]), None,
                                            op.is_equal)
                lo_oh = lo_oh_bufs[g % 2]
                for v in range(LO_POOL):
                    nc.vector.tensor_scalar(lo_oh[:, :, v, :], lo[:],
                                            float(LO_VALS[v]), None,
                                            op.is_equal)
                for v in range(LO_POOL, LO_ACT):
                    nc.gpsimd.tensor_scalar(lo_oh[:, :, v, :], lo[:],
                                            float(LO_VALS[v]), None,
                                            op.is_equal)
                pending.append((g, lo, lo_oh, hi_oh))
                if len(pending) > 1:
                    _flush(pending.pop(0))

            while pending:
                _flush(pending.pop(0))

            w2 = cpool.tile([LO_W + 1, HI_W, CLS], dt.float16)
            nc.sync.dma_start(
                w2[:], w2_d.ap().rearrange("p (m c) -> p m c", m=HI_W))
            bias = cpool.tile([CLS, 1], dt.float32)
            nc.sync.dma_start(bias[:], bias_d[:])
            ps2 = ps2pool.tile([CLS, n_batches], dt.float32)
            for h in range(HI_W):
                nc.tensor.matmul(ps2[:], w2[:, h, :], cnt[:, :, h],
                                 start=(h == 0), stop=(h == HI_W - 1))
            out = cpool.tile([CLS, n_batches], dt.float32)
            nc.vector.tensor_scalar(out[:], ps2[:], 1.0 / N, bias[:],
                                    op.mult, op.add)
            nc.sync.dma_start(y_d[:], out[:])

    nc.compile()
    return nc


def _aux_inputs(W, b):
    # bin(l, h) = 64*q0 + 8*q1 + q2 with q0 = h>>1, q1 = 4*(h&1) + l//8,
    # q2 = l % 8
    ih = np.arange(HI_W)
    il = np.arange(LO_W)
    q0 = ih[:, None] // 2
    b1 = ih[:, None] % 2
    q1 = 4 * b1 + il[None, :] // 8
    q2 = il[None, :] % 8
    binidx = 64 * q0 + 8 * q1 + q2            # [HI_W, LO_W]
    wmap = W[:, binidx]                       # [CLS, HI_W, LO_W]
    w2 = np.zeros((LO_W + 1, HI_W, CLS), np.float32)
    for l in range(LO_W):
        scale = 0.5 if l >= LO_ACT else 1.0
        w2[l] = scale * wmap[:, :, l].T
    # ones-row: sum over +- rows of W/2
    w2[LO_W] = 0.5 * wmap[:, :, LO_ACT:].sum(-1).T
    w2 = np.ascontiguousarray(w2).astype(np.float16).reshape(
        LO_W + 1, HI_W * CLS)
    bias = np.asarray(b, dtype=np.float32).reshape(CLS, 1)
    # act biases: -lo_val for the +- rows, then +0.5 for the Sign pass
    actb = np.zeros((128, LO_W - LO_ACT + 1), np.float32)
    for v in range(LO_ACT, LO_W):
        actb[:, v - LO_ACT] = -float(LO_VALS[v])
    actb[:, LO_W - LO_ACT] = 0.5
    return w2, bias, actb


def kernel(x, W, b):
    from concourse.bass_utils import run_bass_kernel_spmd

    x = np.asarray(x, dtype=np.float32)
    W = np.asarray(W, dtype=np.float32)
    b = np.asarray(b, dtype=np.float32)

    if BPC not in _CACHE:
        _CACHE[BPC] = _build(BPC)
    nc = _CACHE[BPC]

    w2, bias, actb = _aux_inputs(W, b)
    shards = x.reshape(NCORES, BPC, 128, PJ, 3).transpose(0, 2, 1, 3, 4)
    in_maps = [
        {"x": np.ascontiguousarray(shards[i]), "w2": w2, "bias": bias,
         "actb": actb}
        for i in range(NCORES)
    ]
    res = run_bass_kernel_spmd(nc, in_maps, list(range(NCORES)))
    return np.concatenate(
        [np.asarray(res.results[i]["y"]).T for i in range(NCORES)],
        axis=0).astype(np.float32)
